# revision 1
# baseline (speedup 1.0000x reference)
"""MemNet (scatter_memory) Trainium2 kernel.

Model (per batch row b):
  memory   = emb[context_x[b]]                    # [L, D] gather
  v_aspect = masked-mean(emb[target_x[b]])        # [D]
  v_loc    = 1 - |pos - target_loc[b]| / context_len[b]
  3 hops of: scores = tanh((memory*v_loc) @ w_mem + vec@w_vec + b)
             alpha  = masked softmax;  vec = alpha @ (memory*v_loc) + vec@lin_w+lin_b
  logits   = vec @ out_w + out_b

Sharding: data-parallel over batch, 32 rows per core on 8 cores; the fp16
embedding table is replicated (stays in DRAM, rows are fetched by indirect
DMA gather).

Per-core layout: the 32x512 (b,l) pairs are flattened to 16384 rows and
stored in SBUF as [128 partitions, 128 chunk-columns, 300] fp16 (chunk c
holds flat rows c*128..c*128+128, so b = c//4, l = (c%4)*128 + p).
Content scores use a fused multiply+reduce per chunk; attention contraction
runs on the tensor engine as 128 accumulating [K=128, M=32] x [K=128, N=300]
matmuls whose stationary operand is a block-diagonal weight matrix built
with a single strided copy per hop. Softmax needs no max-subtraction
(scores = tanh(..) are in [-1, 1]); denominators come from two small
matmuls (ones-reduction over partitions, then group-sum over chunks).
"""

import numpy as np

import concourse.bass as bass
import concourse.bacc as bacc
import concourse.mybir as mybir
import concourse.tile as tile
from concourse import bass_utils

N_CORES = 8
B, L, T, V, D, C = 256, 512, 5, 50000, 300, 3
N_HOPS = 3
BP = B // N_CORES          # 32 batch rows per core
P = 128                    # partitions
NCH = (BP * L) // P        # 128 chunk columns
CPB = L // P               # 4 chunks per batch row
NGRP = 16                  # gather groups (<=1024 idxs per dma_gather)
GW = NCH // NGRP           # 32 chunk columns per gather group
DK = [128, 128, 44]        # D split for K-contractions
DOF = [0, 128, 256]
TCOL = (BP * T + P - 1) // P  # 2 columns of gathered target rows
EPAD = 384                 # padded row length (768B, 256B-aligned)
U_PAD = 16768              # fixed local-table rows (>= 16384+160)

F16 = mybir.dt.float16
I16 = mybir.dt.int16
F32 = mybir.dt.float32
I32 = mybir.dt.int32


def _free_ap(ap, dims):
    """Replace the free dims of an AP (keep partition dim)."""
    return bass.AP(ap.tensor, ap.offset, [list(ap.ap[0])] + [list(d) for d in dims])


def _bcast_p(ap, n):
    """Broadcast a [1, ...] AP across n partitions (partition step 0)."""
    return bass.AP(ap.tensor, ap.offset, [[0, n]] + [list(d) for d in ap.ap[1:]])


STAGE = "full"
SCORE_G0 = False
_SRANK = {"gather": 0, "masks": 1, "va": 2, "score": 3, "full": 9}


def build_module():
    nc = bacc.Bacc("TRN2", target_bir_lowering=False, debug=False,
                   num_devices=N_CORES)

    emb_d = nc.dram_tensor("emb_loc", [U_PAD, EPAD], F16, kind="ExternalInput")
    ctx_idx_d = nc.dram_tensor("ctx_idx16", [P, NCH * P // 16], I16,
                               kind="ExternalInput")
    tgt_idx_d = nc.dram_tensor("tgt_idx16", [P, TCOL * P // 16], I16,
                               kind="ExternalInput")
    pos_d = nc.dram_tensor("pos_h", [P, NCH], F32, kind="ExternalInput")
    loc_d = nc.dram_tensor("loc_bc", [P, NCH], F32, kind="ExternalInput")
    len_d = nc.dram_tensor("len_bc", [P, NCH], F32, kind="ExternalInput")
    auxp_d = nc.dram_tensor("aux_p", [P, 4], F32, kind="ExternalInput")
    linw_d = nc.dram_tensor("lin_w_h", [P, 3 * 384], F16, kind="ExternalInput")
    outw_d = nc.dram_tensor("out_w_h", [P, 3 * C], F16, kind="ExternalInput")
    linb_d = nc.dram_tensor("lin_b_h", [P, 3], F32, kind="ExternalInput")
    outb_d = nc.dram_tensor("out_b_h", [C, 1], F32, kind="ExternalInput")
    attnb_d = nc.dram_tensor("attn_b_h", [1, 1], F32, kind="ExternalInput")
    wmem_d = nc.dram_tensor("w_mem_h", [P, D], F16, kind="ExternalInput")
    wvec_d = nc.dram_tensor("w_vec_h", [P, 3], F16, kind="ExternalInput")
    ssel_d = nc.dram_tensor("ssel_h", [P, BP], F16, kind="ExternalInput")
    gsel_d = nc.dram_tensor("gsel_h", [P, BP], F16, kind="ExternalInput")
    ones_d = nc.dram_tensor("ones_h", [P, 1], F16, kind="ExternalInput")
    onesr_d = nc.dram_tensor("ones_r", [1, P], F16, kind="ExternalInput")
    id32_d = nc.dram_tensor("id32_h", [BP, BP], F32, kind="ExternalInput")

    out_d = nc.dram_tensor("logits_t", [C, BP], F32, kind="ExternalOutput")

    mult = mybir.AluOpType.mult
    addop = mybir.AluOpType.add
    sub = mybir.AluOpType.subtract
    is_lt = mybir.AluOpType.is_lt
    AF = mybir.ActivationFunctionType

    with tile.TileContext(nc) as tc:
        with (
            tc.tile_pool(name="sb", bufs=1) as sb,
            tc.tile_pool(name="sc", bufs=2) as scr,
            tc.tile_pool(name="ps", bufs=1, space="PSUM") as ps,
            tc.tile_pool(name="ps2", bufs=2, space="PSUM") as ps2,
        ):
            # ---- persistent SBUF tiles ----
            idx_sb = sb.tile([P, NCH * P // 16], I16, tag="idx")
            tgti_sb = sb.tile([P, TCOL * P // 16], I16, tag="tgti")
            mem_sb = [sb.tile([P, GW, EPAD], F16, tag=f"mem{g}", name=f"mem{g}")
                      for g in range(NGRP)]
            tgtr_sb = sb.tile([P, TCOL, EPAD], F16, tag="tgtr")
            pos_sb = sb.tile([P, NCH], F32, tag="pos")
            loc_sb = sb.tile([P, NCH], F32, tag="locbc")
            len_sb = sb.tile([P, NCH], F32, tag="lenbc")
            auxp_sb = sb.tile([P, 4], F32, tag="auxp")
            linw_sb = sb.tile([P, 3, 384], F16, tag="linw")
            outw_sb = sb.tile([P, 3, C], F16, tag="outw")
            linb_sb = sb.tile([P, 3], F32, tag="linb")
            outb_sb = sb.tile([C, 1], F32, tag="outb")
            attnb_sb = sb.tile([1, 1], F32, tag="attnb")
            wmem_sb = sb.tile([P, D], F16, tag="wmem")
            wvec_sb = sb.tile([P, 3], F16, tag="wvec")
            ssel_sb = sb.tile([P, BP], F16, tag="ssel")
            gsel_sb = sb.tile([P, BP], F16, tag="gsel")
            ones_sb = sb.tile([P, 1], F16, tag="ones")
            onesr_sb = sb.tile([1, P], F16, tag="onesr")
            id32_sb = sb.tile([BP, BP], F32, tag="id32")

            mscore = sb.tile([P, NCH], F32, tag="mscore")
            msv = sb.tile([P, NCH], F32, tag="msv")
            vloc = sb.tile([P, NCH], F32, tag="vloc")
            cmask = sb.tile([P, NCH], F32, tag="cmask")
            cv = sb.tile([P, NCH], F16, tag="cv")
            lenr = sb.tile([P, NCH], F32, tag="lenr")
            tmask = sb.tile([P, TCOL], F32, tag="tmask")
            a0 = sb.tile([P, BP, TCOL], F16, tag="a0")
            tlenr = sb.tile([BP, 1], F32, tag="tlenr")
            va_sb = sb.tile([BP, D], F32, tag="va")
            abuf = sb.tile([P, NCH, BP], F16, tag="abuf")
            vecT_a = sb.tile([P, 3, BP], F16, tag="vecTa", name="vecT_a")
            vecT_b = sb.tile([P, 3, BP], F16, tag="vecTb", name="vecT_b")
            sc_f = sb.tile([P, NCH], F32, tag="scf")
            e_m = sb.tile([P, NCH], F16, tag="em")
            svec4 = sb.tile([1, NCH], F16, tag="svec4")
            cs_sb = sb.tile([P, 1], F16, tag="cs")
            rden = sb.tile([BP, 1], F32, tag="rden")
            attn_sb = sb.tile([BP, D], F32, tag="attnsb")
            lg_sb = sb.tile([C, BP], F32, tag="lg")

            # ---- input DMAs ----
            nc.sync.dma_start(idx_sb[:], ctx_idx_d.ap())
            nc.sync.dma_start(tgti_sb[:], tgt_idx_d.ap())
            nc.sync.dma_start(pos_sb[:], pos_d.ap())
            nc.sync.dma_start(loc_sb[:], loc_d.ap())
            nc.sync.dma_start(len_sb[:], len_d.ap())
            nc.sync.dma_start(auxp_sb[:], auxp_d.ap())
            nc.sync.dma_start(linw_sb[:], linw_d.ap())
            nc.sync.dma_start(outw_sb[:], outw_d.ap())
            nc.sync.dma_start(linb_sb[:], linb_d.ap())
            nc.sync.dma_start(outb_sb[:], outb_d.ap())
            nc.sync.dma_start(attnb_sb[:], attnb_d.ap())
            nc.sync.dma_start(wmem_sb[:], wmem_d.ap())
            nc.sync.dma_start(wvec_sb[:], wvec_d.ap())
            nc.sync.dma_start(ssel_sb[:], ssel_d.ap())
            nc.sync.dma_start(gsel_sb[:], gsel_d.ap())
            nc.sync.dma_start(ones_sb[:], ones_d.ap())
            nc.sync.dma_start(onesr_sb[:], onesr_d.ap())
            nc.sync.dma_start(id32_sb[:], id32_d.ap())

            # ---- gathers (InstDMAGatherAnt, the dominant DMA) ----
            nc.gpsimd.dma_gather(
                out_ap=tgtr_sb[:], in_ap=emb_d.ap(), idxs_ap=tgti_sb[:],
                num_idxs=TCOL * P, num_idxs_reg=TCOL * P, elem_size=EPAD)
            NIG = GW * P  # idxs per gather group
            for g in range(NGRP):
                nc.gpsimd.dma_gather(
                    out_ap=mem_sb[g][:], in_ap=emb_d.ap(),
                    idxs_ap=idx_sb[:, g * (NIG // 16):(g + 1) * (NIG // 16)],
                    num_idxs=NIG, num_idxs_reg=NIG, elem_size=EPAD)

            rank = _SRANK[STAGE]
            if rank == 0:
                nc.vector.tensor_copy(out=lg_sb[:], in_=mem_sb[0][0:C, 0, 0:BP])
                nc.sync.dma_start(out_d.ap(), lg_sb[:])
            if rank >= 1:
                    # ---- location model + masks ----
                nc.vector.reciprocal(lenr[:], len_sb[:])
                # dist = |pos - loc|
                nc.vector.tensor_tensor(out=vloc[:], in0=pos_sb[:],
                                        in1=loc_sb[:], op=sub)
                nc.scalar.activation(vloc[:], vloc[:], AF.Abs)
                # vloc = 1 - dist/len
                nc.vector.scalar_tensor_tensor(out=vloc[:], in0=vloc[:], scalar=-1.0,
                                               in1=lenr[:], op0=mult, op1=mult)
                nc.vector.tensor_scalar_add(vloc[:], vloc[:], 1.0)
                nc.vector.tensor_tensor(out=cmask[:], in0=pos_sb[:],
                                        in1=len_sb[:], op=is_lt)
                nc.vector.tensor_tensor(out=cv[:], in0=cmask[:], in1=vloc[:], op=mult)

                # zero the block-diagonal weight buffer once
                nc.vector.memset(abuf[:], 0.0)

            if rank == 1:
                nc.vector.tensor_copy(out=lg_sb[:], in_=cv[0:C, 0:BP])
                nc.sync.dma_start(out_d.ap(), lg_sb[:])
            if rank >= 2:
                    # ---- v_aspect ----
                nc.vector.tensor_tensor(out=tmask[:], in0=auxp_sb[:, 1:3],
                                        in1=auxp_sb[:, 0:1].to_broadcast([P, TCOL]),
                                        op=is_lt)
                va_ps = ps.tile([BP, D], F32, tag="acc300", space="PSUM")
                for j in range(TCOL):
                    nc.vector.tensor_scalar_mul(a0[:, :, j], ssel_sb[:],
                                                tmask[:, j:j + 1])
                    nc.tensor.matmul(va_ps[:], lhsT=a0[:, :, j],
                                     rhs=tgtr_sb[:, j, 0:D],
                                     start=(j == 0), stop=(j == TCOL - 1))
                nc.vector.reciprocal(tlenr[:], auxp_sb[0:BP, 0:1])
                nc.vector.tensor_scalar_mul(va_sb[:], va_ps[:], tlenr[:])

                # vecT0 = v_aspect transposed into [d-part, 3, b]
                for k in range(3):
                    kk = DK[k]
                    t_ps = ps2.tile([P, BP], F32, tag="atT", space="PSUM")
                    nc.tensor.transpose(t_ps[:kk, :], va_sb[:, DOF[k]:DOF[k] + kk],
                                        id32_sb[:])
                    nc.vector.tensor_copy(out=vecT_a[:kk, k, :], in_=t_ps[:kk, :])

            if rank == 2:
                nc.sync.dma_start(out_d.ap(), va_sb[0:C, 0:BP])
            if rank >= 3:
                    # ---- content scores: mscore[p,c] = mem[p,c,:] . w_mem ----
                for c in range(NCH):
                    g, cc = divmod(c, GW)
                    if SCORE_G0:
                        g, cc = 0, c % GW
                    st = scr.tile([P, D], F16, tag="sctmp", bufs=4)
                    nc.vector.scalar_tensor_tensor(
                        out=st[:], in0=mem_sb[g][:, cc, 0:D], scalar=1.0,
                        in1=wmem_sb[:], op0=mult, op1=mult,
                        accum_out=mscore[:, c:c + 1])
                nc.vector.tensor_tensor(out=msv[:], in0=mscore[:], in1=vloc[:],
                                        op=mult)

            if rank == 3:
                nc.sync.dma_start(out_d.ap(), mscore[0:C, 0:BP])
            if rank >= 9:
                    # PE warm-up (HAM): dummy matmuls gated on the last gather
                wu_ps = ps2.tile([BP, 384], F32, tag="psmall", space="PSUM",
                                 name="wu_ps")
                wu_sb = scr.tile([BP, 1], F32, tag="wu_sb")
                for w in range(8):
                    nc.tensor.matmul(
                        wu_ps[:], lhsT=abuf[:, 0, :],
                        rhs=mem_sb[NGRP - 1][:, GW - 1, :],
                        start=True, stop=True)
                nc.vector.tensor_copy(out=wu_sb[:], in_=wu_ps[:, 0:1])
                # ---- hops ----
                for h in range(N_HOPS):
                    vcur = vecT_a if h % 2 == 0 else vecT_b
                    vnxt = vecT_b if h % 2 == 0 else vecT_a
                    # svec = vec @ w_vec  (+ attn_b)
                    svec_ps = ps2.tile([1, BP], F32, tag="psmall", space="PSUM")
                    for k in range(3):
                        kk = DK[k]
                        nc.tensor.matmul(svec_ps[:], lhsT=wvec_sb[:kk, k:k + 1],
                                         rhs=vcur[:kk, k, :],
                                         start=(k == 0), stop=(k == 2))
                    # broadcast per-b -> per-chunk-column (c = 4b + r)
                    sv_in = bass.AP(svec_ps[:].tensor, svec_ps[:].offset,
                                    [list(svec_ps[:].ap[0]), [1, BP], [0, CPB]])
                    nc.vector.tensor_scalar_add(svec4[:], sv_in,
                                                attnb_sb[0:1, 0:1])
                    # linear path (needs only previous vec): emit early so PE
                    # chews on it while the score chain finishes on DVE/ACT
                    lin_ps = []
                    for k in range(3):
                        kk = DK[k]
                        lp = ps2.tile([P, BP], F32, tag="linT", space="PSUM",
                                      name=f"lin_ps{k}", bufs=3)
                        for kx in range(3):
                            kkx = DK[kx]
                            nc.tensor.matmul(
                                lp[:kk, :],
                                lhsT=linw_sb[:kkx, kx, DOF[k]:DOF[k] + kk],
                                rhs=vcur[:kkx, kx, :],
                                start=(kx == 0), stop=(kx == 2))
                        lin_ps.append(lp)
                    # broadcast svec4 across partitions with a K=1 matmul
                    sv_bc = ps2.tile([P, NCH], F32, tag="psmall", space="PSUM",
                                     name="sv_bc")
                    nc.tensor.matmul(sv_bc[:], lhsT=onesr_sb[:], rhs=svec4[:],
                                     start=True, stop=True)
                    # scores = exp(tanh(msv + svec)) ; masked
                    nc.vector.tensor_tensor(out=sc_f[:], in0=msv[:],
                                            in1=sv_bc[:], op=addop)
                    nc.scalar.activation(sc_f[:], sc_f[:], AF.Tanh)
                    nc.scalar.activation(sc_f[:], sc_f[:], AF.Exp)
                    nc.vector.tensor_tensor(out=e_m[:], in0=sc_f[:], in1=cmask[:],
                                            op=mult)
                    # denominator: per-chunk column sums, then group by b
                    cs_ps = ps2.tile([P, 1], F32, tag="psmall", space="PSUM")
                    nc.tensor.matmul(cs_ps[:], lhsT=e_m[:], rhs=ones_sb[:],
                                     start=True, stop=True)
                    nc.vector.tensor_copy(out=cs_sb[:], in_=cs_ps[:])
                    dn_ps = ps2.tile([BP, 1], F32, tag="psmall", space="PSUM")
                    nc.tensor.matmul(dn_ps[:], lhsT=gsel_sb[:], rhs=cs_sb[:],
                                     start=True, stop=True)
                    nc.vector.reciprocal(rden[:], dn_ps[:])
                    # attention weights (alpha * v_loc, un-normalized) scattered
                    # into the block-diagonal stationary buffer:
                    # chunk c -> abuf[:, c, c//4]
                    out_ap = _free_ap(abuf[:], [[CPB * BP + 1, BP], [BP, CPB]])
                    in_q = [[CPB, BP], [1, CPB]]
                    nc.vector.tensor_tensor(out=out_ap,
                                            in0=_free_ap(sc_f[:], in_q),
                                            in1=_free_ap(cv[:], in_q), op=mult)
                    # attention: 128 accumulating matmuls
                    attn_ps = ps.tile([BP, D], F32, tag="acc300", space="PSUM")
                    for c in range(NCH):
                        g, cc = divmod(c, GW)
                        nc.tensor.matmul(attn_ps[:], lhsT=abuf[:, c, :],
                                         rhs=mem_sb[g][:, cc, 0:D],
                                         start=(c == 0), stop=(c == NCH - 1))
                    nc.vector.tensor_scalar_mul(attn_sb[:], attn_ps[:], rden[:])
                    # vec_next^T = (lin_w^T vecT + lin_b) + attn^T, per d-chunk
                    for k in range(3):
                        kk = DK[k]
                        at_ps = ps2.tile([P, BP], F32, tag="atT", space="PSUM")
                        nc.tensor.transpose(at_ps[:kk, :],
                                            attn_sb[:, DOF[k]:DOF[k] + kk],
                                            id32_sb[:])
                        # avoid two PSUM reads in one DVE op: stage lin first
                        tmpv = scr.tile([P, BP], F32, tag="tmpv")
                        nc.vector.tensor_scalar_add(tmpv[:kk, :],
                                                    lin_ps[k][:kk, :],
                                                    linb_sb[:kk, k:k + 1])
                        nc.vector.tensor_tensor(out=vnxt[:kk, k, :],
                                                in0=tmpv[:kk, :],
                                                in1=at_ps[:kk, :], op=addop)

            if rank >= 9:
                # ---- output projection ----
                vfin = vecT_a if N_HOPS % 2 == 0 else vecT_b
                lg_ps = ps2.tile([C, BP], F32, tag="psmall", space="PSUM")
                for k in range(3):
                    kk = DK[k]
                    nc.tensor.matmul(lg_ps[:], lhsT=outw_sb[:kk, k, :],
                                     rhs=vfin[:kk, k, :],
                                     start=(k == 0), stop=(k == 2))
                nc.vector.tensor_scalar_add(lg_sb[:], lg_ps[:], outb_sb[:])
                nc.sync.dma_start(out_d.ap(), lg_sb[:])

    nc.compile()
    return nc


def _wrap16(flat):
    """dma_gather index layout: [128, n/16], replicated over 16-row groups."""
    n = flat.shape[0]
    w = flat.reshape(n // 16, 16).T.astype(np.int16)   # [16, n/16]
    return np.ascontiguousarray(np.tile(w, (8, 1)))    # [128, n/16]


def make_core_inputs(context_x, context_len, target_x, target_len, target_loc,
                     emb16, shared):
    """Per-core input dict. context_x etc are the 32-row shards (numpy).

    The embedding table is sharded per core by index compaction: each core
    receives only the (unique) rows its shard references, padded to 384
    columns (768B, a dma_gather-legal element size), plus int16 local
    indices in the wrapped dma_gather layout.
    """
    flat = np.ascontiguousarray(context_x, dtype=np.int64).reshape(-1)
    tflat = np.zeros(P * TCOL, np.int64)
    tflat[:BP * T] = np.ascontiguousarray(target_x.T, dtype=np.int64).reshape(-1)
    allidx = np.concatenate([flat, tflat])
    uniq, inv = np.unique(allidx, return_inverse=True)
    assert uniq.shape[0] <= U_PAD
    emb_loc = np.zeros((U_PAD, EPAD), np.float16)
    emb_loc[:uniq.shape[0], :D] = emb16[uniq]
    ctx_idx = _wrap16(inv[:flat.shape[0]])
    tgt_idx = _wrap16(inv[flat.shape[0]:])
    cidx = np.arange(NCH) // CPB
    loc_bc = np.broadcast_to(target_loc[cidx].astype(np.float32),
                             (P, NCH)).copy()
    len_bc = np.broadcast_to(context_len[cidx].astype(np.float32),
                             (P, NCH)).copy()
    aux_p = np.zeros((P, 4), np.float32)
    aux_p[:, 0] = target_len[np.arange(P) % BP]
    aux_p[:, 1] = np.arange(P) // BP
    aux_p[:, 2] = (P // BP) + np.arange(P) // BP  # t for j=1: 4 + p//32
    aux_p[:, 3] = np.arange(P)
    d = dict(shared)
    d.update(emb_loc=emb_loc, ctx_idx16=ctx_idx, tgt_idx16=tgt_idx,
             loc_bc=loc_bc, len_bc=len_bc, aux_p=aux_p)
    return d


def make_shared_inputs(emb, attn_w, attn_b, lin_w, lin_b, out_w, out_b):
    pos_h = ((np.arange(NCH)[None, :] % CPB) * P
             + np.arange(P)[:, None]).astype(np.float32)
    lin_w_pad = np.zeros((384, 384), np.float16)
    lin_w_pad[:D, :D] = lin_w.astype(np.float16)
    lin_w_h = np.ascontiguousarray(
        lin_w_pad.reshape(3, P, 384).transpose(1, 0, 2).reshape(P, 3 * 384))
    out_w_pad = np.zeros((384, C), np.float16)
    out_w_pad[:D] = out_w.astype(np.float16)
    out_w_h = np.ascontiguousarray(
        out_w_pad.reshape(3, P, C).transpose(1, 0, 2).reshape(P, 3 * C))
    lin_b_pad = np.zeros((384,), np.float32)
    lin_b_pad[:D] = lin_b
    lin_b_h = np.ascontiguousarray(lin_b_pad.reshape(3, P).T)
    w_vec_pad = np.zeros((384,), np.float16)
    w_vec_pad[:D] = attn_w[D:, 0].astype(np.float16)
    w_vec_h = np.ascontiguousarray(w_vec_pad.reshape(3, P).T)
    ssel = (np.arange(P)[:, None] % BP == np.arange(BP)[None, :])
    gsel = (np.arange(P)[:, None] // CPB == np.arange(BP)[None, :])
    return dict(
        pos_h=pos_h,
        lin_w_h=lin_w_h,
        out_w_h=out_w_h,
        lin_b_h=lin_b_h,
        out_b_h=out_b.astype(np.float32).reshape(C, 1),
        attn_b_h=attn_b.astype(np.float32).reshape(1, 1),
        w_mem_h=np.broadcast_to(attn_w[:D, 0].astype(np.float16),
                                (P, D)).copy(),
        w_vec_h=w_vec_h,
        ssel_h=ssel.astype(np.float16),
        gsel_h=gsel.astype(np.float16),
        ones_h=np.ones((P, 1), np.float16),
        ones_r=np.ones((1, P), np.float16),
        id32_h=np.eye(BP, dtype=np.float32),
    )


_module_cache = {}


def get_module():
    if "nc" not in _module_cache:
        _module_cache["nc"] = build_module()
    return _module_cache["nc"]


def kernel(**inputs):
    emb16 = np.ascontiguousarray(inputs["emb"].astype(np.float16))
    shared = make_shared_inputs(
        np.asarray(inputs["emb"]), np.asarray(inputs["attn_w"]),
        np.asarray(inputs["attn_b"]), np.asarray(inputs["lin_w"]),
        np.asarray(inputs["lin_b"]), np.asarray(inputs["out_w"]),
        np.asarray(inputs["out_b"]))
    in_maps = []
    for k in range(N_CORES):
        s = slice(k * BP, (k + 1) * BP)
        in_maps.append(make_core_inputs(
            np.asarray(inputs["context_x"])[s],
            np.asarray(inputs["context_len"])[s],
            np.asarray(inputs["target_x"])[s],
            np.asarray(inputs["target_len"])[s],
            np.asarray(inputs["target_loc"])[s],
            emb16, shared))
    nc = get_module()
    res = bass_utils.run_bass_kernel_spmd(nc, in_maps,
                                          core_ids=list(range(N_CORES)))
    out = np.concatenate([res.results[k]["logits_t"].T
                          for k in range(N_CORES)], axis=0)
    return out.astype(np.float32)



# revision 7
# speedup vs baseline: 2.0920x; 2.0920x over previous
"""MemNet (scatter_memory) Trainium2 kernel, v2.

Model (per batch row b):
  memory   = emb[context_x[b]]                    # [L, D] gather
  v_aspect = masked-mean(emb[target_x[b]])        # [D]
  v_loc    = 1 - |pos - target_loc[b]| / context_len[b]
  3 hops of: scores = tanh((memory*v_loc) @ w_mem + vec@w_vec + b)
             alpha  = masked softmax;  vec = alpha @ (memory*v_loc) + vec@lin_w+lin_b
  logits   = vec @ out_w + out_b

Sharding: data-parallel over batch, 32 rows per core on 8 cores; the fp16
embedding table is index-compacted per core and fetched by indirect DMA
gather.

Key structure (vs v1):
- The content score emb@w_mem is a pure weight transform; it is appended
  as column 300 of the gathered rows (inside the 768B alignment padding),
  so scores arrive with the gather for free.
- Attention contraction runs transposed on the PE: per (chunk, d-slice),
  stationary = mem[128, DK], moving = the chunk's alpha column [128, 1],
  accumulating over each batch row's 4 chunks directly into vec^T layout
  [DK, b] in PSUM. Cost ~ 1 cycle per matmul (output free size 1).
- v_loc/cmask are host-side index-derived tensors; hop 1 (whose alpha
  depends only on v_aspect) is pipelined per gather group so its attention
  runs under the gather phase; the softmax denominator accumulates per
  group in an open PSUM matmul group.
"""

import numpy as np

import concourse.bass as bass
import concourse.bacc as bacc
import concourse.mybir as mybir
import concourse.tile as tile
from concourse import bass_utils

N_CORES = 8
B, L, T, V, D, C = 256, 512, 5, 50000, 300, 3
N_HOPS = 3
BP = B // N_CORES          # 32 batch rows per core
P = 128                    # partitions
NCH = (BP * L) // P        # 128 chunk columns; chunk c: b=c//4, l=(c%4)*128+p
CPB = L // P               # 4 chunks per batch row
NGRP = 16                  # gather groups (<=1024 idxs per dma_gather)
GW = NCH // NGRP           # chunk columns per gather group (8)
DK = [128, 128, 44]        # D split across PSUM partition chunks
DOF = [0, 128, 256]
TCOL = (BP * T + P - 1) // P  # 2 columns of gathered target rows
EPAD = 384                 # padded row length in fp16 (768B, 256B-aligned)
U_PAD = 16768              # fixed local-table rows (>= 16384+160)

F16 = mybir.dt.float16
I16 = mybir.dt.int16
F32 = mybir.dt.float32

# auxf (f32) column layout
AF_VLOC = 0            # [0:128)   vloc
AF_TLEN = 128          # target_len per partition (p % 32)
AF_T0 = 129            # target t-index for j=0 (p // 32)
AF_T1 = 130            # target t-index for j=1 (4 + p // 32)
AF_ID32 = 131          # [131:163) id32 (rows 0:32)
AF_LINB = 163          # [163:166) lin_b d-chunks
AF_OUTB = 166          # out_b (rows 0:3)
AF_ATTNB = 167         # attn_b (row 0)
AF_CMASK = 168         # [168:296) cmask
AF_CV = 296            # [296:424) cmask * vloc
AF_N = 424

# aux16 (f16) column layout
A6_GSEL = 0            # [0:32)   gsel: chunk c (partition) -> b
A6_SSEL = 32           # [32:64)  ssel: target row p -> b = p % 32
A6_WVEC = 64           # [64:67)  w_vec d-chunks
A6_ONES = 67           # ones column
A6_ONESR = 68          # [68:196) ones row (partition 0)
A6_OUTW = 196          # [196:205) out_w  [d-part, 3 k, C]
A6_N = 205


def _ap2d(tile_ap, col_off, stride, n):
    """2D AP over a 3D tile: partition dim + one strided free dim."""
    return bass.AP(tile_ap.tensor, tile_ap.offset + col_off,
                   [list(tile_ap.ap[0]), [stride, n]])


def _row_rep4(ap2):
    """[1, BP] row AP -> [1, BP, CPB] with the inner dim broadcast (step 0),
    so column c = 4*b + r reads value b."""
    return bass.AP(ap2.tensor, ap2.offset,
                   [list(ap2.ap[0]), [1, BP], [0, CPB]])


def build_module():
    nc = bacc.Bacc("TRN2", target_bir_lowering=False, debug=False,
                   num_devices=N_CORES)

    emb_d = nc.dram_tensor("emb_loc", [U_PAD, EPAD], F16, kind="ExternalInput")
    ctx_idx_d = nc.dram_tensor("ctx_idx16", [P, NCH * P // 16], I16,
                               kind="ExternalInput")
    tgt_idx_d = nc.dram_tensor("tgt_idx16", [P, TCOL * P // 16], I16,
                               kind="ExternalInput")
    auxf_d = nc.dram_tensor("auxf_h", [P, AF_N], F32, kind="ExternalInput")
    aux16_d = nc.dram_tensor("aux16_h", [P, A6_N], F16, kind="ExternalInput")
    linw_d = nc.dram_tensor("lin_w_h", [P, 3 * 384], F16, kind="ExternalInput")

    out_d = nc.dram_tensor("logits_t", [C, BP], F32, kind="ExternalOutput")

    mult = mybir.AluOpType.mult
    addop = mybir.AluOpType.add
    is_lt = mybir.AluOpType.is_lt
    AF = mybir.ActivationFunctionType

    with tile.TileContext(nc) as tc:
        with (
            tc.tile_pool(name="sb", bufs=1) as sb,
            tc.tile_pool(name="sc", bufs=4) as scr,
            tc.tile_pool(name="ps", bufs=1, space="PSUM") as ps,
            tc.tile_pool(name="ps3", bufs=2, space="PSUM") as ps3,
        ):
            # ---- persistent SBUF tiles ----
            tgti_sb = sb.tile([P, TCOL * P // 16], I16, tag="tgti")
            idx_sb = sb.tile([P, NCH * P // 16], I16, tag="idx")
            auxf_sb = sb.tile([P, AF_N], F32, tag="auxf")
            aux16_sb = sb.tile([P, A6_N], F16, tag="aux16")
            linw_sb = sb.tile([P, 3, 384], F16, tag="linw")
            mem_sb = [sb.tile([P, GW, EPAD], F16, tag=f"mem{g}", name=f"mem{g}")
                      for g in range(NGRP)]
            tgtr_sb = sb.tile([P, TCOL, EPAD], F16, tag="tgtr")

            tmask = sb.tile([P, TCOL], F32, tag="tmask")
            a0 = sb.tile([P, BP, TCOL], F16, tag="a0")
            tlenr = sb.tile([BP, 1], F32, tag="tlenr")
            va_sb = sb.tile([BP, D], F32, tag="va")
            vecT_a = sb.tile([P, 3, BP], F16, tag="vecTa", name="vecT_a")
            vecT_b = sb.tile([P, 3, BP], F16, tag="vecTb", name="vecT_b")
            msv = sb.tile([P, NCH], F32, tag="msv")
            sc_f = sb.tile([P, NCH], F32, tag="scf")
            e_m = sb.tile([P, NCH], F16, tag="em")
            aw = sb.tile([P, NCH], F16, tag="aw")
            aw1 = [sb.tile([P, GW], F16, tag=f"aw1_{g}", name=f"aw1_{g}")
                   for g in range(NGRP)]
            svec_sb = sb.tile([1, BP], F16, tag="svec")
            cs_sb = sb.tile([P, 1], F16, tag="cs")
            rdr_sb = sb.tile([1, BP], F16, tag="rdr")
            asm_sb = sb.tile([P, BP], F32, tag="asm")
            lg_sb = sb.tile([C, BP], F32, tag="lg")

            vloc_ap = auxf_sb[:, AF_VLOC:AF_VLOC + NCH]
            cmask_ap = auxf_sb[:, AF_CMASK:AF_CMASK + NCH]
            cv_ap = auxf_sb[:, AF_CV:AF_CV + NCH]
            gsel_ap = aux16_sb[:, A6_GSEL:A6_GSEL + BP]
            ssel_ap = aux16_sb[:, A6_SSEL:A6_SSEL + BP]
            ones_ap = aux16_sb[:, A6_ONES:A6_ONES + 1]
            onesr_ap = aux16_sb[0:1, A6_ONESR:A6_ONESR + P]
            id32_ap = auxf_sb[0:BP, AF_ID32:AF_ID32 + BP]

            # ---- input DMAs (index tensors first so gathers start early) ----
            nc.sync.dma_start(tgti_sb[:], tgt_idx_d.ap())
            nc.sync.dma_start(idx_sb[:], ctx_idx_d.ap())
            nc.sync.dma_start(auxf_sb[:], auxf_d.ap())
            nc.sync.dma_start(aux16_sb[:], aux16_d.ap())
            nc.sync.dma_start(linw_sb[:], linw_d.ap())

            # ---- gathers ----
            nc.gpsimd.dma_gather(
                out_ap=tgtr_sb[:], in_ap=emb_d.ap(), idxs_ap=tgti_sb[:],
                num_idxs=TCOL * P, num_idxs_reg=TCOL * P, elem_size=EPAD)
            NIG = GW * P  # idxs per gather group
            for g in range(NGRP):
                nc.gpsimd.dma_gather(
                    out_ap=mem_sb[g][:], in_ap=emb_d.ap(),
                    idxs_ap=idx_sb[:, g * (NIG // 16):(g + 1) * (NIG // 16)],
                    num_idxs=NIG, num_idxs_reg=NIG, elem_size=EPAD)

            # ---- v_aspect -> vecT_a ----
            nc.vector.tensor_tensor(
                out=tmask[:], in0=auxf_sb[:, AF_T0:AF_T0 + TCOL],
                in1=auxf_sb[:, AF_TLEN:AF_TLEN + 1].to_broadcast([P, TCOL]),
                op=is_lt)
            va_ps = ps.tile([BP, D], F32, tag="acc300", space="PSUM")
            for j in range(TCOL):
                nc.vector.tensor_scalar_mul(a0[:, :, j], ssel_ap,
                                            tmask[:, j:j + 1])
                nc.tensor.matmul(va_ps[:], lhsT=a0[:, :, j],
                                 rhs=tgtr_sb[:, j, 0:D],
                                 start=(j == 0), stop=(j == TCOL - 1))
            nc.vector.reciprocal(tlenr[:], auxf_sb[0:BP, AF_TLEN:AF_TLEN + 1])
            nc.vector.tensor_scalar_mul(va_sb[:], va_ps[:], tlenr[:])
            for k in range(3):
                kk = DK[k]
                t_ps = ps3.tile([P, BP], F32, tag="psmall", space="PSUM")
                nc.tensor.transpose(t_ps[:kk, :], va_sb[:, DOF[k]:DOF[k] + kk],
                                    id32_ap)
                nc.vector.tensor_copy(out=vecT_a[:kk, k, :], in_=t_ps[:kk, :])

            def lin_mms(vcur, lin_ps):
                for k in range(3):
                    kk = DK[k]
                    for kx in range(3):
                        kkx = DK[kx]
                        nc.tensor.matmul(
                            lin_ps[:kk, k, :],
                            lhsT=linw_sb[:kkx, kx, DOF[k]:DOF[k] + kk],
                            rhs=vcur[:kkx, kx, :],
                            start=(kx == 0), stop=(kx == 2))

            def svec_bc(vcur):
                """svec = vec @ w_vec + attn_b, broadcast to [P, NCH]."""
                svec_ps = ps3.tile([1, BP], F32, tag="psmall", space="PSUM")
                for k in range(3):
                    kk = DK[k]
                    nc.tensor.matmul(svec_ps[:],
                                     lhsT=aux16_sb[:kk, A6_WVEC + k:A6_WVEC + k + 1],
                                     rhs=vcur[:kk, k, :],
                                     start=(k == 0), stop=(k == 2))
                nc.vector.tensor_scalar_add(svec_sb[:], svec_ps[:],
                                            auxf_sb[0:1, AF_ATTNB:AF_ATTNB + 1])
                svbc_ps = ps.tile([P, NCH], F32, tag="svbc", space="PSUM")
                nc.tensor.matmul(svbc_ps[:], lhsT=onesr_ap,
                                 rhs=_row_rep4(svec_sb[:]),
                                 start=True, stop=True)
                return svbc_ps

            def attn_mms(attn_ps, alpha_col, c):
                """3 accumulating matmuls: attn^T[:, b] += mem_c^T-slices @ alpha."""
                g, cc = divmod(c, GW)
                b, r = divmod(c, CPB)
                for k in range(3):
                    kk = DK[k]
                    nc.tensor.matmul(
                        attn_ps[:kk, k, b:b + 1],
                        lhsT=mem_sb[g][:, cc, DOF[k]:DOF[k] + DK[k]],
                        rhs=alpha_col,
                        start=(r == 0), stop=(r == CPB - 1))

            def denom_tail(dn_ps):
                """reciprocal + broadcast of the softmax denominator."""
                with nc.allow_low_precision(reason="fp16 1/denom, rel 5e-4"):
                    nc.vector.reciprocal(rdr_sb[:], dn_ps[:])
                rd_ps = ps.tile([P, BP], F32, tag="rdbc", space="PSUM")
                nc.tensor.matmul(rd_ps[:], lhsT=onesr_ap, rhs=rdr_sb[:],
                                 start=True, stop=True)
                return rd_ps

            def assemble(attn_ps, rd_ps, lin_ps, vnxt):
                for k in range(3):
                    kk = DK[k]
                    nc.vector.tensor_tensor(out=asm_sb[:kk, :],
                                            in0=attn_ps[:kk, k, :],
                                            in1=rd_ps[:kk, :], op=mult)
                    nc.vector.scalar_tensor_tensor(
                        out=vnxt[:kk, k, :], in0=lin_ps[:kk, k, :],
                        scalar=auxf_sb[:kk, AF_LINB + k:AF_LINB + k + 1],
                        in1=asm_sb[:kk, :], op0=addop, op1=addop)

            # ======== hop 1 (pipelined per gather group) ========
            lin_ps = ps.tile([P, 3, BP], F32, tag="accL", space="PSUM")
            lin_mms(vecT_a, lin_ps)
            svbc_ps = svec_bc(vecT_a)
            attn_ps = ps.tile([P, 3, BP], F32, tag="accA", space="PSUM")
            for g in range(NGRP):
                gs = g * GW
                score_ap = _ap2d(mem_sb[g][:], D, EPAD, GW)
                nc.vector.tensor_tensor(out=msv[:, gs:gs + GW], in0=score_ap,
                                        in1=vloc_ap[:, gs:gs + GW], op=mult)
                st = scr.tile([P, GW], F32, tag="st", bufs=4)
                nc.vector.tensor_tensor(out=st[:], in0=msv[:, gs:gs + GW],
                                        in1=svbc_ps[:, gs:gs + GW], op=addop)
                nc.scalar.activation(st[:], st[:], AF.Tanh)
                nc.scalar.activation(st[:], st[:], AF.Exp)
                nc.vector.tensor_tensor(out=e_m[:, gs:gs + GW], in0=st[:],
                                        in1=cmask_ap[:, gs:gs + GW], op=mult)
                nc.vector.tensor_tensor(out=aw1[g][:], in0=st[:],
                                        in1=cv_ap[:, gs:gs + GW], op=mult)
                for cc in range(GW):
                    attn_mms(attn_ps, aw1[g][:, cc:cc + 1], gs + cc)
            cs_ps = ps3.tile([P, 1], F32, tag="psmall", space="PSUM")
            nc.tensor.matmul(cs_ps[:], lhsT=e_m[:], rhs=ones_ap,
                             start=True, stop=True)
            nc.vector.tensor_copy(out=cs_sb[:], in_=cs_ps[:])
            dn_ps = ps3.tile([1, BP], F32, tag="psmall", space="PSUM")
            nc.tensor.matmul(dn_ps[:], lhsT=cs_sb[:], rhs=gsel_ap,
                             start=True, stop=True)
            rd_ps = denom_tail(dn_ps)
            assemble(attn_ps, rd_ps, lin_ps, vecT_b)

            # ======== hops 2..N ========
            for h in range(1, N_HOPS):
                vcur = vecT_b if h % 2 == 1 else vecT_a
                vnxt = vecT_a if h % 2 == 1 else vecT_b
                lin_ps = ps.tile([P, 3, BP], F32, tag="accL", space="PSUM")
                lin_mms(vcur, lin_ps)
                svbc_ps = svec_bc(vcur)
                nc.vector.tensor_tensor(out=sc_f[:], in0=msv[:],
                                        in1=svbc_ps[:], op=addop)
                nc.scalar.activation(sc_f[:], sc_f[:], AF.Tanh)
                nc.scalar.activation(sc_f[:], sc_f[:], AF.Exp)
                nc.vector.tensor_tensor(out=e_m[:], in0=sc_f[:],
                                        in1=cmask_ap, op=mult)
                nc.vector.tensor_tensor(out=aw[:], in0=sc_f[:],
                                        in1=cv_ap, op=mult)
                attn_ps = ps.tile([P, 3, BP], F32, tag="accA", space="PSUM")
                cs_ps = ps3.tile([P, 1], F32, tag="psmall", space="PSUM")
                dn_ps = ps3.tile([1, BP], F32, tag="psmall", space="PSUM")
                # denominator ops interleave with the attention stream so the
                # PE never stalls on the DVE copy/reciprocal chain
                nc.tensor.matmul(cs_ps[:], lhsT=e_m[:], rhs=ones_ap,
                                 start=True, stop=True)
                nc.vector.tensor_copy(out=cs_sb[:], in_=cs_ps[:])
                for c in range(NCH // 3):
                    attn_mms(attn_ps, aw[:, c:c + 1], c)
                nc.tensor.matmul(dn_ps[:], lhsT=cs_sb[:], rhs=gsel_ap,
                                 start=True, stop=True)
                for c in range(NCH // 3, 2 * NCH // 3):
                    attn_mms(attn_ps, aw[:, c:c + 1], c)
                rd_ps = denom_tail(dn_ps)
                for c in range(2 * NCH // 3, NCH):
                    attn_mms(attn_ps, aw[:, c:c + 1], c)
                assemble(attn_ps, rd_ps, lin_ps, vnxt)

            # ---- output projection ----
            vfin = vecT_b if N_HOPS % 2 == 1 else vecT_a
            lg_ps = ps3.tile([C, BP], F32, tag="psmall", space="PSUM")
            for k in range(3):
                kk = DK[k]
                nc.tensor.matmul(
                    lg_ps[:],
                    lhsT=aux16_sb[:kk, A6_OUTW + k * C:A6_OUTW + (k + 1) * C],
                    rhs=vfin[:kk, k, :], start=(k == 0), stop=(k == 2))
            nc.vector.tensor_scalar_add(lg_sb[:], lg_ps[:],
                                        auxf_sb[0:C, AF_OUTB:AF_OUTB + 1])
            nc.sync.dma_start(out_d.ap(), lg_sb[:])

    nc.compile()
    return nc


def _wrap16(flat):
    """dma_gather index layout: [128, n/16], replicated over 16-row groups."""
    n = flat.shape[0]
    w = flat.reshape(n // 16, 16).T.astype(np.int16)   # [16, n/16]
    return np.ascontiguousarray(np.tile(w, (8, 1)))    # [128, n/16]


def make_core_inputs(context_x, context_len, target_x, target_len, target_loc,
                     emb16, shared):
    """Per-core input dict. context_x etc are the 32-row shards (numpy).

    The embedding table is sharded per core by index compaction: each core
    receives only the (unique) rows its shard references, padded to 384
    columns (768B, a dma_gather-legal element size) with the precomputed
    content score emb@w_mem at column 300, plus int16 local indices in the
    wrapped dma_gather layout.
    """
    score16 = shared["_score16"]
    flat = np.ascontiguousarray(context_x, dtype=np.int64).reshape(-1)
    tflat = np.zeros(P * TCOL, np.int64)
    tflat[:BP * T] = np.ascontiguousarray(target_x.T, dtype=np.int64).reshape(-1)
    allidx = np.concatenate([flat, tflat])
    uniq, inv = np.unique(allidx, return_inverse=True)
    assert uniq.shape[0] <= U_PAD
    emb_loc = np.zeros((U_PAD, EPAD), np.float16)
    emb_loc[:uniq.shape[0], :D] = emb16[uniq]
    emb_loc[:uniq.shape[0], D] = score16[uniq]
    ctx_idx = _wrap16(inv[:flat.shape[0]])
    tgt_idx = _wrap16(inv[flat.shape[0]:])

    # host-side location model per (p, c): b = c//4, l = (c%4)*128 + p
    cidx = np.arange(NCH) // CPB
    pos = ((np.arange(NCH)[None, :] % CPB) * P
           + np.arange(P)[:, None]).astype(np.float64)
    loc_b = target_loc[cidx].astype(np.float64)[None, :]
    len_b = context_len[cidx].astype(np.float64)[None, :]
    vloc = 1.0 - np.abs(pos - loc_b) / len_b
    cmask = (pos < len_b).astype(np.float64)

    auxf = np.zeros((P, AF_N), np.float32)
    auxf[:, AF_VLOC:AF_VLOC + NCH] = vloc
    auxf[:, AF_TLEN] = target_len[np.arange(P) % BP]
    auxf[:, AF_T0] = np.arange(P) // BP
    auxf[:, AF_T1] = (P // BP) + np.arange(P) // BP
    auxf[:BP, AF_ID32:AF_ID32 + BP] = np.eye(BP)
    auxf[:, AF_LINB:AF_LINB + 3] = shared["_linb3"]
    auxf[:C, AF_OUTB] = shared["_outb"]
    auxf[0, AF_ATTNB] = shared["_attnb"]
    auxf[:, AF_CMASK:AF_CMASK + NCH] = cmask
    auxf[:, AF_CV:AF_CV + NCH] = cmask * vloc

    d = dict(aux16_h=shared["aux16_h"], lin_w_h=shared["lin_w_h"])
    d.update(emb_loc=emb_loc, ctx_idx16=ctx_idx, tgt_idx16=tgt_idx,
             auxf_h=auxf)
    return d


def make_shared_inputs(emb, attn_w, attn_b, lin_w, lin_b, out_w, out_b):
    lin_w_pad = np.zeros((384, 384), np.float16)
    lin_w_pad[:D, :D] = lin_w.astype(np.float16)
    lin_w_h = np.ascontiguousarray(
        lin_w_pad.reshape(3, P, 384).transpose(1, 0, 2).reshape(P, 3 * 384))

    aux16 = np.zeros((P, A6_N), np.float16)
    aux16[:, A6_GSEL:A6_GSEL + BP] = (
        np.arange(P)[:, None] // CPB == np.arange(BP)[None, :])
    aux16[:, A6_SSEL:A6_SSEL + BP] = (
        np.arange(P)[:, None] % BP == np.arange(BP)[None, :])
    w_vec_pad = np.zeros((384,), np.float16)
    w_vec_pad[:D] = attn_w[D:, 0].astype(np.float16)
    aux16[:, A6_WVEC:A6_WVEC + 3] = w_vec_pad.reshape(3, P).T
    aux16[:, A6_ONES] = 1.0
    aux16[0, A6_ONESR:A6_ONESR + P] = 1.0
    out_w_pad = np.zeros((384, C), np.float16)
    out_w_pad[:D] = out_w.astype(np.float16)
    aux16[:, A6_OUTW:A6_OUTW + 3 * C] = (
        out_w_pad.reshape(3, P, C).transpose(1, 0, 2).reshape(P, 3 * C))

    lin_b_pad = np.zeros((384,), np.float32)
    lin_b_pad[:D] = lin_b
    score16 = (np.asarray(emb, np.float64)
               @ np.asarray(attn_w[:D, 0], np.float64)).astype(np.float16)
    return dict(
        lin_w_h=lin_w_h,
        aux16_h=aux16,
        _linb3=np.ascontiguousarray(lin_b_pad.reshape(3, P).T),
        _outb=out_b.astype(np.float32),
        _attnb=np.float32(attn_b[0]),
        _score16=score16,
    )


_module_cache = {}


def get_module():
    if "nc" not in _module_cache:
        _module_cache["nc"] = build_module()
    return _module_cache["nc"]


def kernel(**inputs):
    emb16 = np.ascontiguousarray(inputs["emb"].astype(np.float16))
    shared = make_shared_inputs(
        np.asarray(inputs["emb"]), np.asarray(inputs["attn_w"]),
        np.asarray(inputs["attn_b"]), np.asarray(inputs["lin_w"]),
        np.asarray(inputs["lin_b"]), np.asarray(inputs["out_w"]),
        np.asarray(inputs["out_b"]))
    in_maps = []
    for k in range(N_CORES):
        s = slice(k * BP, (k + 1) * BP)
        in_maps.append(make_core_inputs(
            np.asarray(inputs["context_x"])[s],
            np.asarray(inputs["context_len"])[s],
            np.asarray(inputs["target_x"])[s],
            np.asarray(inputs["target_len"])[s],
            np.asarray(inputs["target_loc"])[s],
            emb16, shared))
    nc = get_module()
    res = bass_utils.run_bass_kernel_spmd(nc, in_maps,
                                          core_ids=list(range(N_CORES)))
    out = np.concatenate([res.results[k]["logits_t"].T
                          for k in range(N_CORES)], axis=0)
    return out.astype(np.float32)
